# revision 14
# baseline (speedup 1.0000x reference)
"""LorentzTransformer Trainium2 kernel (v2 — dense-PE-stream rewrite).

Full inputs in, full output out. Sharding: 8 cores = 2 batches x 4 head
groups (4 heads / 256 channels each). Host pre-transposes x and packs
x/Wq/Wk/Wv into one [D, 1792] fp16 blob so each contraction chunk k is a
single DMA; host sums the 4 head-group partials per batch (fp16 partials).

Per-core pipeline (fp16 PE datapath, fp32 PSUM accumulation):
  warmup matmuls spin the PE from t=0 so the p-state ramp (0.65 -> 2.4 GHz
  after 3us continuous) completes during the input DMA.
  phase 1: k-chunk streaming — per k one DMA lands [x | wq | wk | wv] and
  the PE runs 8 matmuls accumulating Q/K for all (t, qc) groups in 8 PSUM
  banks (weights loaded once per (k,t), both qc chunks streamed).
  lorentz: combined-norm matmul (4 rows: |Q|^2, |Qt|^2 per head), fast
  reciprocal + sqrt -> sf; per-channel factor via K=2 matmul; Q scaled in
  place with the /sqrt(dh) fold.
  V natural-layout proj (x-tiles as weights) + ones column (softmax denom).
  attention per (t, qc): scoresT[k,q] one kt-step ahead of AV so exp (ACT)
  hides under the PE stream; causal via block skip + triangular mask mul
  (DVE); AV+denominator in one PSUM group; normalize via fast-reciprocal
  (fp16 out) + gpsimd partition_broadcast + mul.
  Wo partial for q-chunk 0 is injected into q-chunk 1's attention stream as
  PE gap filler; outputs stored fp16, host sums partials.
"""

import numpy as np

from concourse import bacc
import concourse.tile as tile
import concourse.mybir as mybir
from concourse.bass_utils import run_bass_kernel_spmd

B, L, D, H = 2, 1024, 1024, 16
DH = D // H  # 64
ALPHA = 0.25
SCALE = float(np.sqrt(DH))  # 8.0
HPC = 4          # heads per core
DPC = HPC * DH   # 256 channels per core
N_CORES = 8
P = 128
NQC = L // 512   # q chunks of 512
NKT = L // P     # k tiles of 128
CW = L + 3 * DPC  # 1792: packed chunk width: x(1024) wq(256) wk(256) wv(256)
CB = 456          # const blob cols: mask(128) nb(72) sp(256)

FP = mybir.dt.float32
FPC = mybir.dt.float16
NPC = np.float16

EXP = mybir.ActivationFunctionType.Exp
SQRT = mybir.ActivationFunctionType.Sqrt


def _build_program(debug=False):
    nc = bacc.Bacc("TRN2", target_bir_lowering=False)

    xw = nc.dram_tensor("xw", [D, CW], FPC, kind="ExternalInput")
    woT = nc.dram_tensor("woT", [DPC, D], FPC, kind="ExternalInput")
    cst = nc.dram_tensor("cst", [P, CB], FPC, kind="ExternalInput")
    out = nc.dram_tensor("out", [L, D], FPC, kind="ExternalOutput")

    with tile.TileContext(nc) as tc:
        with (
            tc.tile_pool(name="persist", bufs=1) as persist,
            tc.tile_pool(name="work", bufs=4) as work,
            tc.tile_pool(name="expp", bufs=6) as expp,
            tc.tile_pool(name="sm", bufs=8) as smp,
            tc.tile_pool(name="ost", bufs=4) as ost,
        ):
            xw_sb = [persist.tile([P, CW], FPC, tag=f"xw{k}", name=f"xw{k}")
                     for k in range(D // P)]
            wo_sb = persist.tile([P, 2, D], FPC, tag="wo", name="wo")
            cst_sb = persist.tile([P, CB], FPC, tag="cst", name="cst")
            qT = [persist.tile([P, L], FPC, tag=f"qT{t}", name=f"qT{t}") for t in range(2)]
            kT = [persist.tile([P, L], FPC, tag=f"kT{t}", name=f"kT{t}") for t in range(2)]
            v_sb = persist.tile([P, NKT, HPC, DH + 1], FPC, tag="v", name="v")
            aT = [
                [persist.tile([P, 512], FPC, tag=f"aT{t}{qc}", name=f"aT{t}{qc}") for qc in range(NQC)]
                for t in range(2)
            ]
            scratch = persist.tile([P, 512], FPC, tag="scr", name="scr")
            onecol = persist.tile([P, 1], FP, tag="onecol", name="onecol")

            # ---- input DMAs (SP queue, chunk-ordered for streaming) ----
            nc.sync.dma_start(cst_sb[:], cst[:])
            for k in range(D // P):
                nc.sync.dma_start(xw_sb[k][:], xw[k * P : (k + 1) * P, :])
            nc.sync.dma_start(wo_sb[:], woT.rearrange("(o p) n -> p o n", p=P))

            mk = cst_sb[:, 0:P]  # [k,q] lower-tri (k<=q) 0/1

            nc.gpsimd.memset(scratch[:], 0.0)
            nc.vector.memset(onecol[:], 1.0)
            nc.vector.tensor_copy(
                v_sb[:, :, :, DH : DH + 1],
                onecol.to_broadcast([P, NKT, HPC, 1]),
            )

            with tc.tile_pool(name="psQK", bufs=8, space="PSUM") as psQK:
                # PE warmup: p-state ramp needs ~3us of continuous execution
                for w in range(11):
                    wps = psQK.tile([P, 512], FP, tag="qk", name=f"warm{w}")
                    nc.tensor.matmul(
                        wps[:], scratch[:, 0:P], scratch[:], start=True, stop=True
                    )

                # ---- phase 1: Q/K proj, 8 open PSUM groups, k streaming ----
                grp = {}
                for which in range(2):
                    for t in range(2):
                        for qc in range(NQC):
                            grp[(which, t, qc)] = psQK.tile(
                                [P, 512], FP, tag="qk", name=f"g{which}{t}{qc}"
                            )
                for k in range(D // P):
                    xs = xw_sb[k]
                    for which, base in ((0, L), (1, L + DPC)):
                        for t in range(2):
                            wsl = xs[:, base + t * P : base + (t + 1) * P]
                            for qc in range(NQC):
                                nc.tensor.matmul(
                                    grp[(which, t, qc)][:],
                                    wsl,
                                    xs[:, qc * 512 : (qc + 1) * 512],
                                    start=(k == 0),
                                    stop=(k == D // P - 1),
                                )

                # ---- copies out of PSUM (all 8 banks freed before lorentz
                # rotates new tiles into them), then lorentz ----
                for t in range(2):
                    for qc in range(NQC):
                        nc.scalar.copy(
                            qT[t][:, qc * 512 : (qc + 1) * 512], grp[(0, t, qc)][:]
                        )
                        nc.vector.tensor_copy(
                            kT[t][:, qc * 512 : (qc + 1) * 512], grp[(1, t, qc)][:]
                        )
                # ---- lorentz interleaved with V proj (PE gap filler) ----
                sfs = {}

                def sq_mul(t):
                    sq = work.tile([P, L], FPC, tag="sq", name=f"sq{t}")
                    nc.vector.tensor_mul(sq[:], qT[t][:], qT[t][:])
                    return sq

                def lor_pre(t, qc, sq):
                    sl = slice(qc * 512, (qc + 1) * 512)
                    nps = psQK.tile([P, 512], FP, tag="qk", name=f"nps{t}{qc}")
                    nc.tensor.matmul(
                        nps[:2, :],
                        cst_sb[:, P + 4 * t : P + 4 * t + 2],
                        sq[:, sl],
                        start=True,
                        stop=True,
                    )
                    nqs = psQK.tile([P, 512], FP, tag="qk", name=f"nqs{t}{qc}")
                    nc.tensor.matmul(
                        nqs[:2, :],
                        cst_sb[:, P + 4 * t + 2 : P + 4 * t + 4],
                        sq[:, sl],
                        start=True,
                        stop=True,
                    )
                    brcp = smp.tile([2, 512], FP, tag="brcp", name="brcp")
                    nc.vector.reciprocal_approx_fast(brcp[:], nqs[0:2, :])
                    rat = smp.tile([2, 512], FP, tag="rat", name="rat")
                    nc.vector.tensor_mul(rat[:], nps[0:2, :], brcp[:])
                    sf = smp.tile([2, 512], FPC, tag="sf", name=f"sf{t}{qc}")
                    nc.scalar.activation(sf[:], rat[:], SQRT)
                    sfs[(t, qc)] = sf

                def lor_post(t, qc):
                    sl = slice(qc * 512, (qc + 1) * 512)
                    gps = psQK.tile([P, 512], FP, tag="qk", name=f"gp{t}{qc}")
                    nc.tensor.matmul(
                        gps[:],
                        cst_sb[0:2, 200 + 128 * t : 328 + 128 * t],
                        sfs[(t, qc)][:],
                        start=True,
                        stop=True,
                    )
                    fp_sb = smp.tile([P, 512], FPC, tag="fp", name="fp")
                    nc.vector.tensor_scalar_add(fp_sb[:], gps[:], 1.0 / SCALE)
                    nc.vector.tensor_mul(qT[t][:, sl], qT[t][:, sl], fp_sb[:])

                def v_group(lt, pool=None, tag="qk"):
                    vps = (pool or psQK).tile([P, 512], FP, tag=tag, name=f"v{lt}")
                    for k in range(D // P):
                        nc.tensor.matmul(
                            vps[:, :DPC],
                            xw_sb[k][:, lt * P : (lt + 1) * P],
                            xw_sb[k][:, L + 2 * DPC : L + 3 * DPC],
                            start=(k == 0),
                            stop=(k == D // P - 1),
                        )
                    if lt % 2 == 0:
                        nc.vector.tensor_copy(
                            v_sb[:, lt, :, :DH],
                            vps[:, :DPC].rearrange("p (h d) -> p h d", h=HPC),
                        )
                    else:
                        nc.scalar.copy(
                            v_sb[:, lt, :, :DH],
                            vps[:, :DPC].rearrange("p (h d) -> p h d", h=HPC),
                        )

                sq0 = sq_mul(0)
                sq1 = sq_mul(1)
                for t in range(2):
                    for qc in range(NQC):
                        lor_pre(t, qc, sq0 if t == 0 else sq1)
                for lt in range(5):
                    v_group(lt)
                for t in range(2):
                    for qc in range(NQC):
                        lor_post(t, qc)

            # ---- attention + Wo (both head-pairs interleaved per q-chunk
            # so the PE stream stays dense and exp hides under it) ----
            with (
                tc.tile_pool(name="psSC", bufs=4, space="PSUM") as psSC,
                tc.tile_pool(name="psAV", bufs=4, space="PSUM") as psAV,
            ):
                oc_eng = [nc.scalar, nc.vector]
                oc_n = [0]

                def wo_group(lt):
                    qc = lt // 4
                    ps = [
                        psSC.tile([P, 512], FP, tag="sc", name=f"wo{lt}{jc}")
                        for jc in range(2)
                    ]
                    for t2 in range(2):
                        for jc in range(2):
                            nc.tensor.matmul(
                                ps[jc][:],
                                aT[t2][qc][:, (lt % 4) * P : (lt % 4 + 1) * P],
                                wo_sb[:, t2, jc * 512 : (jc + 1) * 512],
                                start=(t2 == 0),
                                stop=(t2 == 1),
                            )
                    for jc in range(2):
                        oc = ost.tile([P, 512], FPC, tag="oc", name="oc")
                        eng = oc_eng[oc_n[0] % 2]
                        oc_n[0] += 1
                        if eng is nc.scalar:
                            eng.copy(oc[:], ps[jc][:])
                        else:
                            eng.tensor_copy(oc[:], ps[jc][:])
                        nc.sync.dma_start(
                            out[lt * P : (lt + 1) * P, jc * 512 : (jc + 1) * 512],
                            oc[:],
                        )

                def attn2(qc, fillers=None):
                    nkt = 4 * qc + 4
                    avs = {
                        (t, hl): psAV.tile(
                            [DH + 1, 512], FP, tag="av", name=f"av{qc}{t}{hl}"
                        )
                        for t in range(2)
                        for hl in range(2)
                    }

                    def emit_av(kt, exs):
                        for t in range(2):
                            for hl in range(2):
                                ex, off = exs[(t, hl)]
                                nc.tensor.matmul(
                                    avs[(t, hl)][:, off:512],
                                    v_sb[:, kt, 2 * t + hl, :],
                                    ex[:, off:512],
                                    start=(kt == 0),
                                    stop=(kt == nkt - 1),
                                )

                    pend = None
                    for kt in range(nkt):
                        off = max(0, (kt - 4 * qc) * P)
                        j = kt - 4 * qc
                        exs = {}
                        for t in range(2):
                            for hl in range(2):
                                base = hl * DH
                                sc = psSC.tile(
                                    [P, 512], FP, tag="sc", name=f"sc{qc}{kt}{t}{hl}"
                                )
                                nc.tensor.matmul(
                                    sc[:, off:512],
                                    kT[t][base : base + DH, kt * P : (kt + 1) * P],
                                    qT[t][
                                        base : base + DH,
                                        qc * 512 + off : (qc + 1) * 512,
                                    ],
                                    start=True,
                                    stop=True,
                                    tile_position=(base, 0),
                                )
                                ex = expp.tile(
                                    [P, 512], FPC, tag="ex", name=f"ex{qc}{kt}{t}{hl}"
                                )
                                nc.scalar.activation(
                                    ex[:, off:512], sc[:, off:512], EXP
                                )
                                if j >= 0:
                                    nc.vector.tensor_mul(
                                        ex[:, j * P : (j + 1) * P],
                                        ex[:, j * P : (j + 1) * P],
                                        mk,
                                    )
                                exs[(t, hl)] = (ex, off)
                        if pend is not None:
                            emit_av(*pend)
                        if fillers is not None and kt in fillers:
                            fillers[kt]()
                        pend = (kt, exs)
                    emit_av(*pend)
                    return avs

                def norm(t, qc, avs):
                    for hl in range(2):
                        av = avs[(t, hl)]
                        den = smp.tile([1, 512], FP, tag="den", name=f"dn{t}{qc}{hl}")
                        nc.vector.tensor_copy(den[:], av[DH : DH + 1, :])
                        rc = smp.tile([1, 512], FP, tag="rc", name=f"rc{t}{qc}{hl}")
                        nc.vector.reciprocal_approx_fast(rc[:], den[:])
                        bc = work.tile([DH, 512], FP, tag="bc", name="bc")
                        nc.gpsimd.partition_broadcast(bc[:], rc[:], channels=DH)
                        nc.vector.tensor_mul(
                            aT[t][qc][hl * DH : (hl + 1) * DH, :],
                            av[:DH, :],
                            bc[:],
                        )

                if debug:
                    avs0 = attn2(0)
                    av_d = ost.tile([DH + 1, 512], FPC, tag="avd", name="avd")
                    nc.scalar.copy(av_d[:], avs0[(0, 0)][:])
                    nc.sync.dma_start(out[640 : 640 + DH + 1, 0:512], av_d[:])
                    norm(0, 0, avs0)
                    qd = ost.tile([P, L], FPC, tag="qd", name="qd")
                    nc.vector.tensor_copy(qd[:], qT[0][:])
                    nc.sync.dma_start(out[0:P, :], qd[:])
                    kd = ost.tile([P, L], FPC, tag="kd", name="kd")
                    nc.vector.tensor_copy(kd[:], kT[0][:])
                    nc.sync.dma_start(out[P : 2 * P, :], kd[:])
                    ad = ost.tile([P, 512], FPC, tag="ad", name="ad")
                    nc.vector.tensor_copy(ad[:], aT[0][0][:])
                    nc.sync.dma_start(out[512 : 512 + P, 0:512], ad[:])
                else:
                    avs0 = attn2(
                        0,
                        {
                            1: lambda: v_group(5, psSC, "sc"),
                            2: lambda: v_group(6, psSC, "sc"),
                            3: lambda: v_group(7, psSC, "sc"),
                        },
                    )
                    norm(0, 0, avs0)
                    norm(1, 0, avs0)
                    avs1 = attn2(
                        1,
                        {
                            2: lambda: wo_group(0),
                            3: lambda: wo_group(1),
                            4: lambda: wo_group(2),
                            5: lambda: wo_group(3),
                        },
                    )
                    norm(0, 1, avs1)
                    norm(1, 1, avs1)
                    for lt in range(4, 8):
                        wo_group(lt)

    nc.compile()
    return nc


_NC = None


def _host_inputs(x, Wq, Wk, Wv, Wo, timelike_mask):
    m_full = np.asarray(timelike_mask).astype(np.float32)
    mt = np.tril(np.ones((P, P), dtype=np.float32)).T.copy()  # [k,q]=1 iff k<=q
    in_maps = []
    for c in range(N_CORES):
        b, g = divmod(c, HPC)
        sl = slice(g * DPC, (g + 1) * DPC)
        m = m_full[sl]  # [256]
        cstb = np.zeros((P, CB), dtype=np.float32)
        cstb[:, 0:P] = mt
        for t in range(2):
            m_t = m[t * P : (t + 1) * P]
            nb = np.zeros((P, 4), dtype=np.float32)
            nb[0:DH, 0] = 1.0
            nb[DH:P, 1] = 1.0
            nb[0:DH, 2] = m_t[0:DH]
            nb[DH:P, 3] = m_t[DH:P]
            cstb[:, P + 4 * t : P + 4 * t + 4] = nb
            coef = -2.0 * ALPHA / SCALE  # -0.0625
            sp = np.zeros((2, P), dtype=np.float32)
            sp[0, 0:DH] = coef * m_t[0:DH]
            sp[1, DH:P] = coef * m_t[DH:P]
            cstb[0:2, 200 + 128 * t : 328 + 128 * t] = sp
        xwb = np.empty((D, CW), dtype=NPC)
        xwb[:, 0:L] = x[b].T
        xwb[:, L : L + DPC] = Wq[sl, :].T
        xwb[:, L + DPC : L + 2 * DPC] = Wk[sl, :].T
        xwb[:, L + 2 * DPC : L + 3 * DPC] = Wv[sl, :].T
        in_maps.append(
            {
                "xw": np.ascontiguousarray(xwb),
                "woT": np.ascontiguousarray(Wo[:, sl].T).astype(NPC),
                "cst": cstb.astype(NPC),
            }
        )
    return in_maps


def kernel(x, Wq, Wk, Wv, Wo, timelike_mask, attn_mask, _trace=False, _debug=False):
    global _NC
    if _NC is None:
        _NC = _build_program(debug=_debug)
    nc = _NC

    x = np.asarray(x, dtype=np.float32)
    Wq, Wk, Wv, Wo = (np.asarray(w, dtype=np.float32) for w in (Wq, Wk, Wv, Wo))
    am = np.asarray(attn_mask, dtype=np.float32).reshape(L, L)
    causal = np.tril(np.ones((L, L), dtype=bool))
    assert np.array_equal(am, np.where(causal, 0.0, -1e9).astype(np.float32)), (
        "kernel hardcodes a causal additive mask"
    )

    in_maps = _host_inputs(x, Wq, Wk, Wv, Wo, timelike_mask)
    res = run_bass_kernel_spmd(
        nc, in_maps, core_ids=list(range(N_CORES)), trace=_trace
    )
    outp = np.stack(
        [
            sum(
                res.results[b * HPC + g]["out"].astype(np.float32)
                for g in range(HPC)
            )
            for b in range(B)
        ]
    )
    kernel.last_results = res
    return outp


# revision 15
# speedup vs baseline: 1.2283x; 1.2283x over previous
"""LorentzTransformer Trainium2 kernel (v2 — dense-PE-stream rewrite).

Full inputs in, full output out. Sharding: 8 cores = 2 batches x 4 head
groups (4 heads / 256 channels each). Host pre-transposes x and packs
x/Wq/Wk/Wv into one [D, 1792] fp16 blob so each contraction chunk k is a
single DMA; host sums the 4 head-group partials per batch (fp16 partials).

Per-core pipeline (fp16 PE datapath, fp32 PSUM accumulation):
  warmup matmuls spin the PE from t=0 so the p-state ramp (0.65 -> 2.4 GHz
  after 3us continuous) completes during the input DMA.
  phase 1: k-chunk streaming — per k one DMA lands [x | wq | wk | wv] and
  the PE runs 8 matmuls accumulating Q/K for all (t, qc) groups in 8 PSUM
  banks (weights loaded once per (k,t), both qc chunks streamed).
  lorentz: combined-norm matmul (4 rows: |Q|^2, |Qt|^2 per head), fast
  reciprocal + sqrt -> sf; per-channel factor via K=2 matmul; Q scaled in
  place with the /sqrt(dh) fold.
  V natural-layout proj (x-tiles as weights) + ones column (softmax denom).
  attention per (t, qc): scoresT[k,q] one kt-step ahead of AV so exp (ACT)
  hides under the PE stream; causal via block skip + triangular mask mul
  (DVE); AV+denominator in one PSUM group; normalize via fast-reciprocal
  (fp16 out) + gpsimd partition_broadcast + mul.
  Wo partial for q-chunk 0 is injected into q-chunk 1's attention stream as
  PE gap filler; outputs stored fp16, host sums partials.
"""

import numpy as np

from concourse import bacc
import concourse.tile as tile
import concourse.mybir as mybir
from concourse.bass_utils import run_bass_kernel_spmd

B, L, D, H = 2, 1024, 1024, 16
DH = D // H  # 64
ALPHA = 0.25
SCALE = float(np.sqrt(DH))  # 8.0
HPC = 4          # heads per core
DPC = HPC * DH   # 256 channels per core
N_CORES = 8
P = 128
NQC = L // 512   # q chunks of 512
NKT = L // P     # k tiles of 128
CW = L + 3 * DPC  # 1792: packed chunk width: x(1024) wq(256) wk(256) wv(256)
CB = 456          # const blob cols: mask(128) nb(72) sp(256)

FP = mybir.dt.float32
FPC = mybir.dt.float16
NPC = np.float16

EXP = mybir.ActivationFunctionType.Exp
SQRT = mybir.ActivationFunctionType.Sqrt


def _build_program(debug=False):
    nc = bacc.Bacc("TRN2", target_bir_lowering=False)

    xw = nc.dram_tensor("xw", [D, CW], FPC, kind="ExternalInput")
    woT = nc.dram_tensor("woT", [DPC, D], FPC, kind="ExternalInput")
    cst = nc.dram_tensor("cst", [P, CB], FPC, kind="ExternalInput")
    out = nc.dram_tensor("out", [L, D], FPC, kind="ExternalOutput")

    with tile.TileContext(nc) as tc:
        with (
            tc.tile_pool(name="persist", bufs=1) as persist,
            tc.tile_pool(name="work", bufs=4) as work,
            tc.tile_pool(name="expp", bufs=6) as expp,
            tc.tile_pool(name="sm", bufs=8) as smp,
            tc.tile_pool(name="ost", bufs=4) as ost,
        ):
            xw_sb = [persist.tile([P, CW], FPC, tag=f"xw{k}", name=f"xw{k}")
                     for k in range(D // P)]
            wo_sb = persist.tile([P, 2, D], FPC, tag="wo", name="wo")
            cst_sb = persist.tile([P, CB], FPC, tag="cst", name="cst")
            qT = [persist.tile([P, L], FPC, tag=f"qT{t}", name=f"qT{t}") for t in range(2)]
            kT = [persist.tile([P, L], FPC, tag=f"kT{t}", name=f"kT{t}") for t in range(2)]
            v_sb = persist.tile([P, NKT, HPC, DH + 1], FPC, tag="v", name="v")
            aT = [
                [persist.tile([P, 512], FPC, tag=f"aT{t}{qc}", name=f"aT{t}{qc}") for qc in range(NQC)]
                for t in range(2)
            ]
            scratch = persist.tile([P, 512], FPC, tag="scr", name="scr")
            onecol = persist.tile([P, 1], FP, tag="onecol", name="onecol")

            # ---- input DMAs (SP queue, chunk-ordered for streaming) ----
            nc.sync.dma_start(cst_sb[:], cst[:])
            for k in range(D // P):
                nc.sync.dma_start(xw_sb[k][:], xw[k * P : (k + 1) * P, :])
            nc.sync.dma_start(wo_sb[:], woT.rearrange("(o p) n -> p o n", p=P))

            mk = cst_sb[:, 0:P]  # [k,q] lower-tri (k<=q) 0/1

            nc.gpsimd.memset(scratch[:], 0.0)
            nc.vector.memset(onecol[:], 1.0)
            nc.vector.tensor_copy(
                v_sb[:, :, :, DH : DH + 1],
                onecol.to_broadcast([P, NKT, HPC, 1]),
            )

            with tc.tile_pool(name="psQK", bufs=8, space="PSUM") as psQK:
                # PE warmup: p-state ramp needs ~3us of continuous execution
                for w in range(11):
                    wps = psQK.tile([P, 512], FP, tag="qk", name=f"warm{w}")
                    nc.tensor.matmul(
                        wps[:], scratch[:, 0:P], scratch[:], start=True, stop=True
                    )

                # ---- phase 1: Q/K proj, 8 open PSUM groups, k streaming ----
                grp = {}
                for which in range(2):
                    for t in range(2):
                        for qc in range(NQC):
                            grp[(which, t, qc)] = psQK.tile(
                                [P, 512], FP, tag="qk", name=f"g{which}{t}{qc}"
                            )
                for k in range(D // P):
                    xs = xw_sb[k]
                    for which, base in ((0, L), (1, L + DPC)):
                        for t in range(2):
                            wsl = xs[:, base + t * P : base + (t + 1) * P]
                            for qc in range(NQC):
                                nc.tensor.matmul(
                                    grp[(which, t, qc)][:],
                                    wsl,
                                    xs[:, qc * 512 : (qc + 1) * 512],
                                    start=(k == 0),
                                    stop=(k == D // P - 1),
                                )

                # ---- copies out of PSUM (all 8 banks freed before lorentz
                # rotates new tiles into them), then lorentz ----
                for t in range(2):
                    for qc in range(NQC):
                        nc.scalar.copy(
                            qT[t][:, qc * 512 : (qc + 1) * 512], grp[(0, t, qc)][:]
                        )
                        nc.vector.tensor_copy(
                            kT[t][:, qc * 512 : (qc + 1) * 512], grp[(1, t, qc)][:]
                        )
                # ---- lorentz interleaved with V proj (PE gap filler) ----
                sfs = {}

                def sq_mul(t):
                    sq = work.tile([P, L], FPC, tag="sq", name=f"sq{t}")
                    nc.vector.tensor_mul(sq[:], qT[t][:], qT[t][:])
                    return sq

                def lor_pre(t, qc, sq):
                    sl = slice(qc * 512, (qc + 1) * 512)
                    nps = psQK.tile([P, 512], FP, tag="qk", name=f"nps{t}{qc}")
                    nc.tensor.matmul(
                        nps[:2, :],
                        cst_sb[:, P + 4 * t : P + 4 * t + 2],
                        sq[:, sl],
                        start=True,
                        stop=True,
                    )
                    nqs = psQK.tile([P, 512], FP, tag="qk", name=f"nqs{t}{qc}")
                    nc.tensor.matmul(
                        nqs[:2, :],
                        cst_sb[:, P + 4 * t + 2 : P + 4 * t + 4],
                        sq[:, sl],
                        start=True,
                        stop=True,
                    )
                    brcp = smp.tile([2, 512], FP, tag="brcp", name="brcp")
                    nc.vector.reciprocal_approx_fast(brcp[:], nqs[0:2, :])
                    rat = smp.tile([2, 512], FP, tag="rat", name="rat")
                    nc.vector.tensor_mul(rat[:], nps[0:2, :], brcp[:])
                    sf = smp.tile([2, 512], FPC, tag="sf", name=f"sf{t}{qc}")
                    nc.scalar.activation(sf[:], rat[:], SQRT)
                    sfs[(t, qc)] = sf

                def lor_post(t, qc):
                    sl = slice(qc * 512, (qc + 1) * 512)
                    gps = psQK.tile([P, 512], FP, tag="qk", name=f"gp{t}{qc}")
                    nc.tensor.matmul(
                        gps[:],
                        cst_sb[0:2, 200 + 128 * t : 328 + 128 * t],
                        sfs[(t, qc)][:],
                        start=True,
                        stop=True,
                    )
                    nc.vector.scalar_tensor_tensor(
                        qT[t][:, sl],
                        gps[:],
                        1.0 / SCALE,
                        qT[t][:, sl],
                        mybir.AluOpType.add,
                        mybir.AluOpType.mult,
                    )

                def v_group(lt, pool=None, tag="qk"):
                    vps = (pool or psQK).tile([P, 512], FP, tag=tag, name=f"v{lt}")
                    for k in range(D // P):
                        nc.tensor.matmul(
                            vps[:, :DPC],
                            xw_sb[k][:, lt * P : (lt + 1) * P],
                            xw_sb[k][:, L + 2 * DPC : L + 3 * DPC],
                            start=(k == 0),
                            stop=(k == D // P - 1),
                        )
                    nc.scalar.copy(
                        v_sb[:, lt, :, :DH],
                        vps[:, :DPC].rearrange("p (h d) -> p h d", h=HPC),
                    )

                sq0 = sq_mul(0)
                sq1 = sq_mul(1)
                for t in range(2):
                    for qc in range(NQC):
                        lor_pre(t, qc, sq0 if t == 0 else sq1)
                for lt in range(5):
                    v_group(lt)
                for t in range(2):
                    for qc in range(NQC):
                        lor_post(t, qc)

            # ---- attention + Wo (both head-pairs interleaved per q-chunk
            # so the PE stream stays dense and exp hides under it) ----
            with (
                tc.tile_pool(name="psSC", bufs=4, space="PSUM") as psSC,
                tc.tile_pool(name="psAV", bufs=4, space="PSUM") as psAV,
            ):
                oc_eng = [nc.scalar, nc.vector]
                oc_n = [0]

                def wo_group(lt):
                    qc = lt // 4
                    ps = [
                        psSC.tile([P, 512], FP, tag="sc", name=f"wo{lt}{jc}")
                        for jc in range(2)
                    ]
                    for t2 in range(2):
                        for jc in range(2):
                            nc.tensor.matmul(
                                ps[jc][:],
                                aT[t2][qc][:, (lt % 4) * P : (lt % 4 + 1) * P],
                                wo_sb[:, t2, jc * 512 : (jc + 1) * 512],
                                start=(t2 == 0),
                                stop=(t2 == 1),
                            )
                    for jc in range(2):
                        oc = ost.tile([P, 512], FPC, tag="oc", name="oc")
                        eng = oc_eng[oc_n[0] % 2]
                        oc_n[0] += 1
                        if eng is nc.scalar:
                            eng.copy(oc[:], ps[jc][:])
                        else:
                            eng.tensor_copy(oc[:], ps[jc][:])
                        nc.sync.dma_start(
                            out[lt * P : (lt + 1) * P, jc * 512 : (jc + 1) * 512],
                            oc[:],
                        )

                def attn2(qc, fillers=None):
                    nkt = 4 * qc + 4
                    avs = {
                        (t, hl): psAV.tile(
                            [DH + 1, 512], FP, tag="av", name=f"av{qc}{t}{hl}"
                        )
                        for t in range(2)
                        for hl in range(2)
                    }

                    def emit_av(kt, exs):
                        for t in range(2):
                            for hl in range(2):
                                ex, off = exs[(t, hl)]
                                nc.tensor.matmul(
                                    avs[(t, hl)][:, off:512],
                                    v_sb[:, kt, 2 * t + hl, :],
                                    ex[:, off:512],
                                    start=(kt == 0),
                                    stop=(kt == nkt - 1),
                                )

                    pend = None
                    for kt in range(nkt):
                        off = max(0, (kt - 4 * qc) * P)
                        j = kt - 4 * qc
                        exs = {}
                        for t in range(2):
                            for hl in range(2):
                                base = hl * DH
                                sc = psSC.tile(
                                    [P, 512], FP, tag="sc", name=f"sc{qc}{kt}{t}{hl}"
                                )
                                nc.tensor.matmul(
                                    sc[:, off:512],
                                    kT[t][base : base + DH, kt * P : (kt + 1) * P],
                                    qT[t][
                                        base : base + DH,
                                        qc * 512 + off : (qc + 1) * 512,
                                    ],
                                    start=True,
                                    stop=True,
                                    tile_position=(base, 0),
                                )
                                ex = expp.tile(
                                    [P, 512], FPC, tag="ex", name=f"ex{qc}{kt}{t}{hl}"
                                )
                                nc.scalar.activation(
                                    ex[:, off:512], sc[:, off:512], EXP
                                )
                                if j >= 0:
                                    nc.vector.tensor_mul(
                                        ex[:, j * P : (j + 1) * P],
                                        ex[:, j * P : (j + 1) * P],
                                        mk,
                                    )
                                exs[(t, hl)] = (ex, off)
                        if pend is not None:
                            emit_av(*pend)
                        if fillers is not None and kt in fillers:
                            fillers[kt]()
                        pend = (kt, exs)
                    emit_av(*pend)
                    return avs

                def norm(t, qc, avs):
                    for hl in range(2):
                        av = avs[(t, hl)]
                        den = smp.tile([1, 512], FP, tag="den", name=f"dn{t}{qc}{hl}")
                        nc.vector.tensor_copy(den[:], av[DH : DH + 1, :])
                        rc = smp.tile([1, 512], FP, tag="rc", name=f"rc{t}{qc}{hl}")
                        nc.vector.reciprocal_approx_fast(rc[:], den[:])
                        bc = work.tile([DH, 512], FP, tag="bc", name="bc")
                        nc.gpsimd.partition_broadcast(bc[:], rc[:], channels=DH)
                        nc.vector.tensor_mul(
                            aT[t][qc][hl * DH : (hl + 1) * DH, :],
                            av[:DH, :],
                            bc[:],
                        )

                if debug:
                    avs0 = attn2(0)
                    av_d = ost.tile([DH + 1, 512], FPC, tag="avd", name="avd")
                    nc.scalar.copy(av_d[:], avs0[(0, 0)][:])
                    nc.sync.dma_start(out[640 : 640 + DH + 1, 0:512], av_d[:])
                    norm(0, 0, avs0)
                    qd = ost.tile([P, L], FPC, tag="qd", name="qd")
                    nc.vector.tensor_copy(qd[:], qT[0][:])
                    nc.sync.dma_start(out[0:P, :], qd[:])
                    kd = ost.tile([P, L], FPC, tag="kd", name="kd")
                    nc.vector.tensor_copy(kd[:], kT[0][:])
                    nc.sync.dma_start(out[P : 2 * P, :], kd[:])
                    ad = ost.tile([P, 512], FPC, tag="ad", name="ad")
                    nc.vector.tensor_copy(ad[:], aT[0][0][:])
                    nc.sync.dma_start(out[512 : 512 + P, 0:512], ad[:])
                else:
                    avs0 = attn2(
                        0,
                        {
                            1: lambda: v_group(5, psSC, "sc"),
                            2: lambda: v_group(6, psSC, "sc"),
                            3: lambda: v_group(7, psSC, "sc"),
                        },
                    )
                    norm(0, 0, avs0)
                    norm(1, 0, avs0)
                    avs1 = attn2(
                        1,
                        {
                            2: lambda: wo_group(0),
                            3: lambda: wo_group(1),
                            4: lambda: wo_group(2),
                            5: lambda: wo_group(3),
                        },
                    )
                    norm(0, 1, avs1)
                    norm(1, 1, avs1)
                    for lt in range(4, 8):
                        wo_group(lt)

    nc.compile()
    return nc


_NC = None


def _host_inputs(x, Wq, Wk, Wv, Wo, timelike_mask):
    m_full = np.asarray(timelike_mask).astype(np.float32)
    mt = np.tril(np.ones((P, P), dtype=np.float32)).T.copy()  # [k,q]=1 iff k<=q
    in_maps = []
    for c in range(N_CORES):
        b, g = divmod(c, HPC)
        sl = slice(g * DPC, (g + 1) * DPC)
        m = m_full[sl]  # [256]
        cstb = np.zeros((P, CB), dtype=np.float32)
        cstb[:, 0:P] = mt
        for t in range(2):
            m_t = m[t * P : (t + 1) * P]
            nb = np.zeros((P, 4), dtype=np.float32)
            nb[0:DH, 0] = 1.0
            nb[DH:P, 1] = 1.0
            nb[0:DH, 2] = m_t[0:DH]
            nb[DH:P, 3] = m_t[DH:P]
            cstb[:, P + 4 * t : P + 4 * t + 4] = nb
            coef = -2.0 * ALPHA / SCALE  # -0.0625
            sp = np.zeros((2, P), dtype=np.float32)
            sp[0, 0:DH] = coef * m_t[0:DH]
            sp[1, DH:P] = coef * m_t[DH:P]
            cstb[0:2, 200 + 128 * t : 328 + 128 * t] = sp
        xwb = np.empty((D, CW), dtype=NPC)
        xwb[:, 0:L] = x[b].T
        xwb[:, L : L + DPC] = Wq[sl, :].T
        xwb[:, L + DPC : L + 2 * DPC] = Wk[sl, :].T
        xwb[:, L + 2 * DPC : L + 3 * DPC] = Wv[sl, :].T
        in_maps.append(
            {
                "xw": np.ascontiguousarray(xwb),
                "woT": np.ascontiguousarray(Wo[:, sl].T).astype(NPC),
                "cst": cstb.astype(NPC),
            }
        )
    return in_maps


def kernel(x, Wq, Wk, Wv, Wo, timelike_mask, attn_mask, _trace=False, _debug=False):
    global _NC
    if _NC is None:
        _NC = _build_program(debug=_debug)
    nc = _NC

    x = np.asarray(x, dtype=np.float32)
    Wq, Wk, Wv, Wo = (np.asarray(w, dtype=np.float32) for w in (Wq, Wk, Wv, Wo))
    am = np.asarray(attn_mask, dtype=np.float32).reshape(L, L)
    causal = np.tril(np.ones((L, L), dtype=bool))
    assert np.array_equal(am, np.where(causal, 0.0, -1e9).astype(np.float32)), (
        "kernel hardcodes a causal additive mask"
    )

    in_maps = _host_inputs(x, Wq, Wk, Wv, Wo, timelike_mask)
    res = run_bass_kernel_spmd(
        nc, in_maps, core_ids=list(range(N_CORES)), trace=_trace
    )
    outp = np.stack(
        [
            sum(
                res.results[b * HPC + g]["out"].astype(np.float32)
                for g in range(HPC)
            )
            for b in range(B)
        ]
    )
    kernel.last_results = res
    return outp
